# revision 1
# baseline (speedup 1.0000x reference)
"""Multi-head attention (B=2, H=16, S=2048, D=64) on 8 trn2 NeuronCores.

Sharding: the 32 (b, h) head-units are split 4-per-core (head/data parallel,
no cross-core comms).  Per core, for each head:

  scoresT[k, q] = sum_d K[k, d] Q[q, d] / 8        (PE, contract=64, row-packed 2x)
  pT[k, q]      = exp(scoresT) * keep01T[k, q]     (ACT exp fused w/ scale + psum
                                                    evacuation; DVE fp16 mask mul)
  OT'[m, q]     = sum_k V'[k, m] pT[k, q]          (PE, V' = [V | ones] so row 64
                                                    of OT' is the softmax denom Z)
  out[q, d]     = OT'[d, q] / OT'[64, q]           (host-side: O(S*D) divide +
                                                    transpose while unsharding)

Working in the transposed-score layout means softmax needs no reductions at
all (Z rides along in the PV matmul) and no S x S transposes anywhere.

Host-side (numpy, not on the critical HW path): Q/K are passed pre-transposed
per head as [64, S]; V is passed chunk-interleaved fp16 with the ones column
appended; the shared mask is passed transposed as a 0/1 fp16 matrix.
"""

import numpy as np

import concourse.bass as bass  # noqa: F401  (engine types resolve through nc)
import concourse.mybir as mybir
import concourse.tile as tile
from concourse import bacc
from concourse.bass_utils import run_bass_kernel_spmd

B, H, S, D = 2, 16, 2048, 64
N_CORES = 8
HPC = (B * H) // N_CORES  # heads per core

SQ = 512        # query-block width (one fp32 PSUM bank)
CK = 128        # key-chunk height (PSUM partition dim)
# Key chunks per exp group: 3-bank [128, 1536] PSUM groups maximize the ACT
# call size (per-ACTIVATE overhead is ~0.4us on HW) within the 8-bank budget
# (2x 3-bank qk slots + 2x 1-bank PV accumulators).
GROUPS = [(0, 3), (3, 3), (6, 3), (9, 3), (12, 3), (15, 1)]
HALVES = [(0, 0, 2), (1, 2, 6)]   # (half idx, first group, end group)
VW = D + 2      # V' width: 64 V columns + ones column + pad (66)

f32 = mybir.dt.float32
f16 = mybir.dt.float16
FT = mybir.ActivationFunctionType


def build_nc(hpc=HPC, s=S, loop_n=None, ablate=(), loop_stagger=False):
    """Build the per-core Bass program (identical on all 8 cores).

    loop_n: if set, wrap the whole body in an on-device For_i loop that
    recomputes the same output loop_n times — a perf-measurement rig that
    lets wall-clock deltas between two loop_n values cancel host/RPC
    overheads (this container has no NTFF profile path).

    ablate: perf-debug only — subset of {"qk", "act", "mask", "pv", "tail"}
    to skip emitting, isolating per-engine throughput on HW. Output is
    garbage when non-empty.
    """
    nsq = s // SQ
    nck = s // CK
    groups = [(c0, n) for c0, n in GROUPS if c0 + n <= nck] if nck == 16 else [
        (c, 1) for c in range(nck)]
    ablate = set(ablate)

    nc = bacc.Bacc("TRN2", target_bir_lowering=False, debug=False)

    qt_d = nc.dram_tensor("qt", [hpc, D, s], f16, kind="ExternalInput")
    kt_d = nc.dram_tensor("kt", [hpc, D, s], f16, kind="ExternalInput")
    vp_d = nc.dram_tensor("vp", [hpc, CK, nck * VW], f16, kind="ExternalInput")
    mk_d = nc.dram_tensor("mk", [nsq, CK, nck * SQ], f16, kind="ExternalInput")
    o_d = nc.dram_tensor("o", [hpc, nsq, VW, SQ], f32, kind="ExternalOutput")

    with tile.TileContext(nc) as tc:
        if ablate:
            tc.race_detector_enabled = False
        with (
            tc.tile_pool(name="heads", bufs=hpc) as head_pool,
            tc.tile_pool(name="mask", bufs=nsq) as mask_pool,
            tc.tile_pool(name="pt", bufs=2) as pt_pool,
            tc.tile_pool(name="tail", bufs=2) as tail_pool,
            tc.tile_pool(name="qk_ps", bufs=2, space="PSUM") as qk_pool,
            tc.tile_pool(name="o_ps", bufs=2, space="PSUM") as o_pool,
        ):
            qt_t, kt_t, vp_t = [], [], []
            for h in range(hpc):
                q_t = head_pool.tile([128, s], f16, name=f"qt_sb{h}", tag="qt")
                k_t = head_pool.tile([128, s], f16, name=f"kt_sb{h}", tag="kt")
                v_t = head_pool.tile([CK, nck * VW], f16, name=f"vp_sb{h}", tag="vp")
                # Q^T/K^T live duplicated in both partition halves so the two
                # row-packed K=64 matmuls can run concurrently on the PE.
                nc.sync.dma_start(out=q_t[0:D, :], in_=qt_d[h, :, :])
                nc.sync.dma_start(out=q_t[D:128, :], in_=qt_d[h, :, :])
                nc.sync.dma_start(out=k_t[0:D, :], in_=kt_d[h, :, :])
                nc.sync.dma_start(out=k_t[D:128, :], in_=kt_d[h, :, :])
                nc.sync.dma_start(out=v_t[:, :], in_=vp_d[h, :, :])
                qt_t.append(q_t)
                kt_t.append(k_t)
                vp_t.append(v_t)

            # The whole 0/1 mask fits in SBUF — load it once, outside any
            # measurement loop (saves 8MB of DMA per pass).
            mk_t = {}     # sqb -> mask tile [128, nck*SQ] (chunk-major columns)
            for sqb in range(nsq):
                mk = mask_pool.tile([CK, nck * SQ], f16, name=f"mk_sb{sqb}",
                                    tag="mk")
                nc.sync.dma_start(out=mk[:, :], in_=mk_d[sqb, :, :])
                mk_t[sqb] = mk

            pt_t = {}     # (sqb, h) -> p^T tile [128, nck*SQ] fp16
            o_ps = {}     # (sqb, h) -> PSUM accumulator [VW, SQ]

            def emit_qk_group(sqb, h, c0, n):
                """QK matmuls + exp for chunks [c0, c0+n)."""
                qk = None
                if "qk" not in ablate:
                    qk = qk_pool.tile([128, n * SQ], f32,
                                      name=f"qk_{sqb}_{h}_{c0}", tag="qk",
                                      padded_shape=[128, 3 * SQ])
                for j in range(n):
                    if "qk" in ablate:
                        break
                    c = c0 + j
                    bp = 64 * (j % 2)  # row-group for PE packing
                    nc.tensor.matmul(
                        qk[:, j * SQ:(j + 1) * SQ],
                        lhsT=kt_t[h][bp:bp + D, c * CK:(c + 1) * CK],
                        rhs=qt_t[h][bp:bp + D, sqb * SQ:(sqb + 1) * SQ],
                        start=True,
                        stop=True,
                        tile_position=(bp, 0),
                    )
                pt = pt_t[(sqb, h)]
                lo = c0 * SQ
                hi = (c0 + n) * SQ
                if "act" not in ablate:
                    act_in = qk[:, :] if qk is not None else mk_t[sqb][:, lo:hi]
                    nc.scalar.activation(pt[:, lo:hi], act_in, FT.Exp, scale=0.125)

            def emit_mask(sqb, h, clo, chi):
                """Apply the 0/1 keep-mask to chunk cols [clo, chi) of p^T
                in one DVE pass (per-call overhead is ~0.4us; batch big)."""
                if "mask" in ablate:
                    return
                pt = pt_t[(sqb, h)]
                lo, hi = clo * SQ, chi * SQ
                nc.vector.tensor_tensor(
                    pt[:, lo:hi], pt[:, lo:hi], mk_t[sqb][:, lo:hi],
                    op=mybir.AluOpType.mult,
                )

            def emit_pv(sqb, h, clo, chi):
                """PV matmuls for chunks [clo, chi), accumulating."""
                if "pv" in ablate:
                    return
                pt = pt_t[(sqb, h)]
                if "act" in ablate and "mask" in ablate:
                    pt = mk_t[sqb]  # stand-in written tile for PE-only ablations
                ops = o_ps[(sqb, h)]
                for c in range(clo, chi):
                    nc.tensor.matmul(
                        ops[:, :],
                        lhsT=vp_t[h][:, c * VW:c * VW + VW],
                        rhs=pt[:, c * SQ:(c + 1) * SQ],
                        start=(c == 0),
                        stop=(c == nck - 1),
                    )

            def emit_tail(sqb, h):
                """Evacuate O^T' (unnormalized + Z row) and store."""
                if "tail" in ablate:
                    return
                ops = o_ps[(sqb, h)]
                ot = tail_pool.tile([VW, SQ], f32, name=f"ot_{sqb}_{h}", tag="ot")
                nc.vector.tensor_copy(ot[:, :], ops[:, :])
                nc.sync.dma_start(out=o_d[h, sqb, :, :], in_=ot[:, :])

            # Half-stage software pipeline over (sqb, h, half): half k's
            # QK/exp/mask stream overlaps the PV matmuls of half k-2 (one
            # full stage earlier), so the in-order PE queue never stalls on
            # the ACT/DVE work of the half just emitted.
            ghalves = [(g0, g1) for _, g0, g1 in HALVES]
            if len(groups) != len(GROUPS):  # small-s debug builds: one half
                ghalves = [(0, len(groups))]

            def emit_front(sqb, h, hf):
                if hf == 0:
                    if not ({"act", "mask"} <= ablate):
                        pt_t[(sqb, h)] = pt_pool.tile(
                            [128, nck * SQ], f16, name=f"pt_{sqb}_{h}",
                            tag="pt")
                    else:
                        pt_t[(sqb, h)] = None
                    if "pv" not in ablate:
                        o_ps[(sqb, h)] = o_pool.tile(
                            [VW, SQ], f32, name=f"ops_{sqb}_{h}", tag="ops")
                g0, g1 = ghalves[hf]
                for c0, n in groups[g0:g1]:
                    emit_qk_group(sqb, h, c0, n)
                clo = groups[g0][0]
                chi = (groups[g1 - 1][0] + groups[g1 - 1][1])
                emit_mask(sqb, h, clo, chi)

            def emit_back(sqb, h, hf):
                g0, g1 = ghalves[hf]
                clo = groups[g0][0]
                chi = (groups[g1 - 1][0] + groups[g1 - 1][1])
                emit_pv(sqb, h, clo, chi)
                if hf == len(ghalves) - 1:
                    emit_tail(sqb, h)

            def emit_all():
                halves = [(sqb, h, hf)
                          for sqb in range(nsq) for h in range(hpc)
                          for hf in range(len(ghalves))]
                for k, hv in enumerate(halves):
                    emit_front(*hv)
                    if k >= 2:
                        emit_back(*halves[k - 2])
                for hv in halves[-2:]:
                    emit_back(*hv)

            if loop_n is None:
                emit_all()
            else:
                hints = (mybir.EngineType.PE, mybir.EngineType.Activation,
                         mybir.EngineType.DVE)
                with tc.For_i(0, loop_n, 1, hint_engines=hints,
                              staggered_reset=bool(loop_stagger)):
                    emit_all()

    nc.finalize()
    return nc


def shard_inputs(K, Q, V, mask, hpc=HPC, s=S, n_cores=N_CORES):
    """Full inputs -> per-core in_maps with device-friendly host layouts."""
    nsq = s // SQ
    nck = s // CK
    n_units = n_cores * hpc
    Kf = np.asarray(K, np.float32).reshape(n_units, s, D)
    Qf = np.asarray(Q, np.float32).reshape(n_units, s, D)
    Vf = np.asarray(V, np.float32).reshape(n_units, s, D)
    keepT = (~np.asarray(mask).reshape(s, s)).T  # [k, q], True = attend
    mk_host = np.ascontiguousarray(
        keepT.astype(np.float16)
        .reshape(nck, CK, nsq, SQ)
        .transpose(2, 1, 0, 3)
        .reshape(nsq, CK, nck * SQ)
    )
    in_maps = []
    for c in range(n_cores):
        sl = slice(c * hpc, (c + 1) * hpc)
        qt = np.ascontiguousarray(Qf[sl].transpose(0, 2, 1)).astype(np.float16)
        kt = np.ascontiguousarray(Kf[sl].transpose(0, 2, 1)).astype(np.float16)
        vp = np.zeros((hpc, s, VW), np.float16)
        vp[:, :, :D] = Vf[sl]
        vp[:, :, D] = 1.0
        vp = np.ascontiguousarray(
            vp.reshape(hpc, nck, CK, VW).transpose(0, 2, 1, 3)
            .reshape(hpc, CK, nck * VW)
        )
        in_maps.append({"qt": qt, "kt": kt, "vp": vp, "mk": mk_host})
    return in_maps


_NC_CACHE = {}


def _get_nc():
    if "nc" not in _NC_CACHE:
        _NC_CACHE["nc"] = build_nc()
    return _NC_CACHE["nc"]


def run_sharded(in_maps, trace=False, **kwargs):
    return run_bass_kernel_spmd(
        _get_nc(), in_maps, core_ids=list(range(N_CORES)), trace=trace, **kwargs
    )


def unshard_output(per_core_raw, hpc=HPC, s=S):
    """[hpc, nsq, VW, SQ] raw blocks per core -> [n*hpc, s, D] normalized.

    Row D of each block is the softmax denominator Z; dividing and
    transposing here is O(S*D) host work (same order as unsharding).
    """
    n = len(per_core_raw)
    out = np.empty((n * hpc, s, D), np.float32)
    for c, o in enumerate(per_core_raw):
        ot = o[:, :, :D, :] / o[:, :, D:D + 1, :]   # [hpc, nsq, D, SQ]
        out[c * hpc:(c + 1) * hpc] = (
            ot.transpose(0, 1, 3, 2).reshape(hpc, s, D))
    return out


def assemble_output(results):
    out = unshard_output([results[c]["o"] for c in range(N_CORES)])
    return out.reshape(B, H, S, D)


def kernel(K, Q, V, mask):
    in_maps = shard_inputs(K, Q, V, mask)
    res = run_sharded(in_maps)
    return assemble_output(res.results)



# revision 4
# speedup vs baseline: 1.0439x; 1.0439x over previous
"""Multi-head attention (B=2, H=16, S=2048, D=64) on 8 trn2 NeuronCores.

Sharding: the 32 (b, h) head-units are split 4-per-core (head/data parallel,
no cross-core comms).  Per core, for each head:

  scoresT[k, q] = sum_d K[k, d] Q[q, d] / 8        (PE, contract=64, row-packed 2x)
  pT[k, q]      = exp(scoresT) * keep01T[k, q]     (ACT exp fused w/ scale + psum
                                                    evacuation; DVE fp16 mask mul)
  OT'[m, q]     = sum_k V'[k, m] pT[k, q]          (PE, V' = [V | ones] so row 64
                                                    of OT' is the softmax denom Z)
  out[q, d]     = OT'[d, q] / OT'[64, q]           (host-side: O(S*D) divide +
                                                    transpose while unsharding)

Working in the transposed-score layout means softmax needs no reductions at
all (Z rides along in the PV matmul) and no S x S transposes anywhere.

Host-side (numpy, not on the critical HW path): Q/K are passed pre-transposed
per head as [64, S]; V is passed chunk-interleaved fp16 with the ones column
appended; the shared mask is passed transposed as a 0/1 fp16 matrix.
"""

import numpy as np

import concourse.bass as bass  # noqa: F401  (engine types resolve through nc)
import concourse.mybir as mybir
import concourse.tile as tile
from concourse import bacc
from concourse.bass_utils import run_bass_kernel_spmd

B, H, S, D = 2, 16, 2048, 64
N_CORES = 8
HPC = (B * H) // N_CORES  # heads per core

SQ = 512        # query-block width (one fp32 PSUM bank)
CK = 128        # key-chunk height (PSUM partition dim)
# Key chunks per exp group: 3-bank [128, 1536] PSUM groups maximize the ACT
# call size (per-ACTIVATE overhead is ~0.4us on HW) within the 8-bank budget
# (2x 3-bank qk slots + 2x 1-bank PV accumulators).
GROUPS = [(0, 3), (3, 3), (6, 3), (9, 3), (12, 3), (15, 1)]
HALVES = [(0, 0, 2), (1, 2, 6)]   # (half idx, first group, end group)
VW = D + 2      # V' width: 64 V columns + ones column + pad (66)

f32 = mybir.dt.float32
f16 = mybir.dt.float16
FT = mybir.ActivationFunctionType


def build_nc(hpc=HPC, s=S, loop_n=None, ablate=(), loop_stagger=False):
    """Build the per-core Bass program (identical on all 8 cores).

    loop_n: if set, wrap the whole body in an on-device For_i loop that
    recomputes the same output loop_n times — a perf-measurement rig that
    lets wall-clock deltas between two loop_n values cancel host/RPC
    overheads (this container has no NTFF profile path).

    ablate: perf-debug only — subset of {"qk", "act", "mask", "pv", "tail"}
    to skip emitting, isolating per-engine throughput on HW. Output is
    garbage when non-empty.
    """
    nsq = s // SQ
    nck = s // CK
    groups = [(c0, n) for c0, n in GROUPS if c0 + n <= nck] if nck == 16 else [
        (c, 1) for c in range(nck)]
    ablate = set(ablate)

    nc = bacc.Bacc("TRN2", target_bir_lowering=False, debug=False)

    qt_d = nc.dram_tensor("qt", [hpc, D, s], f16, kind="ExternalInput")
    kt_d = nc.dram_tensor("kt", [hpc, D, s], f16, kind="ExternalInput")
    vp_d = nc.dram_tensor("vp", [hpc, CK, nck * VW], f16, kind="ExternalInput")
    mk_d = nc.dram_tensor("mk", [nsq, CK, nck * SQ], f16, kind="ExternalInput")
    o_d = nc.dram_tensor("o", [hpc, nsq, VW, SQ], f32, kind="ExternalOutput")

    with tile.TileContext(nc) as tc:
        if ablate:
            tc.race_detector_enabled = False
        with (
            tc.tile_pool(name="heads", bufs=hpc) as head_pool,
            tc.tile_pool(name="mask", bufs=nsq) as mask_pool,
            tc.tile_pool(name="pt", bufs=2) as pt_pool,
            tc.tile_pool(name="tail", bufs=2) as tail_pool,
            tc.tile_pool(name="qk_ps", bufs=2, space="PSUM") as qk_pool,
            tc.tile_pool(name="o_ps", bufs=2, space="PSUM") as o_pool,
        ):
            qt_t, kt_t, vp_t = [], [], []
            for h in range(hpc):
                q_t = head_pool.tile([128, s], f16, name=f"qt_sb{h}", tag="qt")
                k_t = head_pool.tile([128, s], f16, name=f"kt_sb{h}", tag="kt")
                v_t = head_pool.tile([CK, nck * VW], f16, name=f"vp_sb{h}", tag="vp")
                # Q^T/K^T live duplicated in both partition halves so the two
                # row-packed K=64 matmuls can run concurrently on the PE.
                nc.sync.dma_start(out=q_t[0:D, :], in_=qt_d[h, :, :])
                nc.sync.dma_start(out=q_t[D:128, :], in_=qt_d[h, :, :])
                nc.sync.dma_start(out=k_t[0:D, :], in_=kt_d[h, :, :])
                nc.sync.dma_start(out=k_t[D:128, :], in_=kt_d[h, :, :])
                nc.sync.dma_start(out=v_t[:, :], in_=vp_d[h, :, :])
                qt_t.append(q_t)
                kt_t.append(k_t)
                vp_t.append(v_t)

            # The whole 0/1 mask fits in SBUF — load it once, outside any
            # measurement loop (saves 8MB of DMA per pass).
            mk_t = {}     # sqb -> mask tile [128, nck*SQ] (chunk-major columns)
            for sqb in range(nsq):
                mk = mask_pool.tile([CK, nck * SQ], f16, name=f"mk_sb{sqb}",
                                    tag="mk")
                nc.sync.dma_start(out=mk[:, :], in_=mk_d[sqb, :, :])
                mk_t[sqb] = mk

            pt_t = {}     # (sqb, h) -> p^T tile [128, nck*SQ] fp16
            o_ps = {}     # (sqb, h) -> PSUM accumulator [VW, SQ]

            def emit_qk_group(sqb, h, c0, n):
                """QK matmuls + exp for chunks [c0, c0+n)."""
                qk = None
                if "qk" not in ablate:
                    qk = qk_pool.tile([128, n * SQ], f32,
                                      name=f"qk_{sqb}_{h}_{c0}", tag="qk",
                                      padded_shape=[128, 3 * SQ])
                for j in range(n):
                    if "qk" in ablate:
                        break
                    c = c0 + j
                    # Row-group alternation must be strict across the WHOLE
                    # chunk sequence (c % 2, not j % 2): only back-to-back
                    # (0, 64) pairs stream concurrently on the PE; a repeated
                    # row-group at a group boundary serializes both matmuls
                    # (measured 359 ns/MM vs 128.5 ns/MM packed).
                    bp = 64 * (c % 2)
                    nc.tensor.matmul(
                        qk[:, j * SQ:(j + 1) * SQ],
                        lhsT=kt_t[h][bp:bp + D, c * CK:(c + 1) * CK],
                        rhs=qt_t[h][bp:bp + D, sqb * SQ:(sqb + 1) * SQ],
                        start=True,
                        stop=True,
                        tile_position=(bp, 0),
                    )
                pt = pt_t[(sqb, h)]
                lo = c0 * SQ
                hi = (c0 + n) * SQ
                if "act" not in ablate:
                    act_in = qk[:, :] if qk is not None else mk_t[sqb][:, lo:hi]
                    nc.scalar.activation(pt[:, lo:hi], act_in, FT.Exp, scale=0.125)

            def emit_mask(sqb, h, clo, chi):
                """Apply the 0/1 keep-mask to chunk cols [clo, chi) of p^T
                in one DVE pass (per-call overhead is ~0.4us; batch big)."""
                if "mask" in ablate:
                    return
                pt = pt_t[(sqb, h)]
                lo, hi = clo * SQ, chi * SQ
                nc.vector.tensor_tensor(
                    pt[:, lo:hi], pt[:, lo:hi], mk_t[sqb][:, lo:hi],
                    op=mybir.AluOpType.mult,
                )

            def emit_pv(sqb, h, clo, chi):
                """PV matmuls for chunks [clo, chi), accumulating."""
                if "pv" in ablate:
                    return
                pt = pt_t[(sqb, h)]
                if "act" in ablate and "mask" in ablate:
                    pt = mk_t[sqb]  # stand-in written tile for PE-only ablations
                ops = o_ps[(sqb, h)]
                for c in range(clo, chi):
                    nc.tensor.matmul(
                        ops[:, :],
                        lhsT=vp_t[h][:, c * VW:c * VW + VW],
                        rhs=pt[:, c * SQ:(c + 1) * SQ],
                        start=(c == 0),
                        stop=(c == nck - 1),
                    )

            def emit_tail(sqb, h):
                """Evacuate O^T' (unnormalized + Z row) and store."""
                if "tail" in ablate:
                    return
                ops = o_ps[(sqb, h)]
                ot = tail_pool.tile([VW, SQ], f32, name=f"ot_{sqb}_{h}", tag="ot")
                nc.vector.tensor_copy(ot[:, :], ops[:, :])
                nc.sync.dma_start(out=o_d[h, sqb, :, :], in_=ot[:, :])

            # Half-stage software pipeline over (sqb, h, half): half k's
            # QK/exp/mask stream overlaps the PV matmuls of half k-2 (one
            # full stage earlier), so the in-order PE queue never stalls on
            # the ACT/DVE work of the half just emitted.
            ghalves = [(g0, g1) for _, g0, g1 in HALVES]
            if len(groups) != len(GROUPS):  # small-s debug builds: one half
                ghalves = [(0, len(groups))]

            def emit_front(sqb, h, hf):
                if hf == 0:
                    if not ({"act", "mask"} <= ablate):
                        pt_t[(sqb, h)] = pt_pool.tile(
                            [128, nck * SQ], f16, name=f"pt_{sqb}_{h}",
                            tag="pt")
                    else:
                        pt_t[(sqb, h)] = None
                    if "pv" not in ablate:
                        o_ps[(sqb, h)] = o_pool.tile(
                            [VW, SQ], f32, name=f"ops_{sqb}_{h}", tag="ops")
                g0, g1 = ghalves[hf]
                for c0, n in groups[g0:g1]:
                    emit_qk_group(sqb, h, c0, n)
                clo = groups[g0][0]
                chi = (groups[g1 - 1][0] + groups[g1 - 1][1])
                emit_mask(sqb, h, clo, chi)

            def emit_back(sqb, h, hf):
                g0, g1 = ghalves[hf]
                clo = groups[g0][0]
                chi = (groups[g1 - 1][0] + groups[g1 - 1][1])
                emit_pv(sqb, h, clo, chi)
                if hf == len(ghalves) - 1:
                    emit_tail(sqb, h)

            def emit_all():
                halves = [(sqb, h, hf)
                          for sqb in range(nsq) for h in range(hpc)
                          for hf in range(len(ghalves))]
                for k, hv in enumerate(halves):
                    emit_front(*hv)
                    if k >= 2:
                        emit_back(*halves[k - 2])
                for hv in halves[-2:]:
                    emit_back(*hv)

            if loop_n is None:
                emit_all()
            else:
                hints = (mybir.EngineType.PE, mybir.EngineType.Activation,
                         mybir.EngineType.DVE)
                with tc.For_i(0, loop_n, 1, hint_engines=hints,
                              staggered_reset=bool(loop_stagger)):
                    emit_all()

    nc.finalize()
    return nc


def shard_inputs(K, Q, V, mask, hpc=HPC, s=S, n_cores=N_CORES):
    """Full inputs -> per-core in_maps with device-friendly host layouts."""
    nsq = s // SQ
    nck = s // CK
    n_units = n_cores * hpc
    Kf = np.asarray(K, np.float32).reshape(n_units, s, D)
    Qf = np.asarray(Q, np.float32).reshape(n_units, s, D)
    Vf = np.asarray(V, np.float32).reshape(n_units, s, D)
    keepT = (~np.asarray(mask).reshape(s, s)).T  # [k, q], True = attend
    mk_host = np.ascontiguousarray(
        keepT.astype(np.float16)
        .reshape(nck, CK, nsq, SQ)
        .transpose(2, 1, 0, 3)
        .reshape(nsq, CK, nck * SQ)
    )
    in_maps = []
    for c in range(n_cores):
        sl = slice(c * hpc, (c + 1) * hpc)
        qt = np.ascontiguousarray(Qf[sl].transpose(0, 2, 1)).astype(np.float16)
        kt = np.ascontiguousarray(Kf[sl].transpose(0, 2, 1)).astype(np.float16)
        vp = np.zeros((hpc, s, VW), np.float16)
        vp[:, :, :D] = Vf[sl]
        vp[:, :, D] = 1.0
        vp = np.ascontiguousarray(
            vp.reshape(hpc, nck, CK, VW).transpose(0, 2, 1, 3)
            .reshape(hpc, CK, nck * VW)
        )
        in_maps.append({"qt": qt, "kt": kt, "vp": vp, "mk": mk_host})
    return in_maps


_NC_CACHE = {}


def _get_nc():
    if "nc" not in _NC_CACHE:
        _NC_CACHE["nc"] = build_nc()
    return _NC_CACHE["nc"]


def run_sharded(in_maps, trace=False, **kwargs):
    return run_bass_kernel_spmd(
        _get_nc(), in_maps, core_ids=list(range(N_CORES)), trace=trace, **kwargs
    )


def unshard_output(per_core_raw, hpc=HPC, s=S):
    """[hpc, nsq, VW, SQ] raw blocks per core -> [n*hpc, s, D] normalized.

    Row D of each block is the softmax denominator Z; dividing and
    transposing here is O(S*D) host work (same order as unsharding).
    """
    n = len(per_core_raw)
    out = np.empty((n * hpc, s, D), np.float32)
    for c, o in enumerate(per_core_raw):
        ot = o[:, :, :D, :] / o[:, :, D:D + 1, :]   # [hpc, nsq, D, SQ]
        out[c * hpc:(c + 1) * hpc] = (
            ot.transpose(0, 1, 3, 2).reshape(hpc, s, D))
    return out


def assemble_output(results):
    out = unshard_output([results[c]["o"] for c in range(N_CORES)])
    return out.reshape(B, H, S, D)


def kernel(K, Q, V, mask):
    in_maps = shard_inputs(K, Q, V, mask)
    res = run_sharded(in_maps)
    return assemble_output(res.results)



# revision 6
# speedup vs baseline: 1.0888x; 1.0430x over previous
"""Multi-head attention (B=2, H=16, S=2048, D=64) on 8 trn2 NeuronCores.

Sharding: the 32 (b, h) head-units are split 4-per-core (head/data parallel,
no cross-core comms).  Per core, for each head:

  scoresT[k, q] = sum_d K[k, d] Q[q, d] / 8        (PE, contract=64, row-packed 2x)
  pT[k, q]      = exp(scoresT) * keep01T[k, q]     (split ACT / DVE, see below)
  OT'[m, q]     = sum_k V'[k, m] pT[k, q]          (PE, V' = [V | ones] so row 64
                                                    of OT' is the softmax denom Z)
  out[q, d]     = OT'[d, q] / OT'[64, q]           (host-side: O(S*D) divide +
                                                    transpose while unsharding)

Working in the transposed-score layout means softmax needs no reductions at
all (Z rides along in the PV matmul) and no S x S transposes anywhere.

exp+mask engine split: ACT's 1 elem/cycle/lane rate makes the exp of all
S*S scores the systemic bottleneck, so the last NDVE k-chunks bypass ACT and
are exponentiated on the DVE with a two-sample Schraudolph bit-trick:
  passA: u1 = int16(scores*A + maskbias)  (scalar_tensor_tensor from PSUM;
         maskbias = B1 for attend / -50000 for masked -> saturates to -32768,
         whose fp16 bit-pattern is -0.0, i.e. the mask is folded in for free)
  passC: u2 = max(u1 + 1536, 0)           (tensor_scalar, 4x mode; masked
         lanes land at exactly +0.0)
  passD: p = f16view(u2) * W + f16view(u1)  (pair-average cancels the primary
         mantissa-interpolation ripple: +-0.8% exp error, ~4e-3 end-to-end)

PE shape rules (measured): QK row-group alternation must be strict (c % 2)
for concurrent streaming of packed contract-64 pairs (128.5 ns/MM); PV runs
as two N=256 matmuls per chunk to hide per-chunk LDWEIGHTS (128.3 ns/MM).

Host-side (numpy, not on the critical HW path): Q/K are passed pre-transposed
per head as [64, S]; V is passed chunk-interleaved fp16 with the ones column
appended; the shared mask is passed transposed as a 0/1 fp16 matrix plus a
Schraudolph bias matrix for the DVE chunks.
"""

import numpy as np

import concourse.bass as bass  # noqa: F401  (engine types resolve through nc)
import concourse.mybir as mybir
import concourse.tile as tile
from concourse import bacc
from concourse.bass_utils import run_bass_kernel_spmd

B, H, S, D = 2, 16, 2048, 64
N_CORES = 8
HPC = (B * H) // N_CORES  # heads per core

SQ = 512        # query-block width (one fp32 PSUM bank)
CK = 128        # key-chunk height (PSUM partition dim)
NDVE = 4        # trailing k-chunks exponentiated on the DVE instead of ACT
# Key chunks per exp group: 3-bank [128, 1536] PSUM groups maximize the ACT
# call size (per-ACTIVATE overhead is ~0.4us on HW) within the 8-bank budget
# (2x 3-bank qk slots + 2x 1-bank PV accumulators).
VW = D + 2      # V' width: 64 V columns + ones column + pad (66)

# Two-sample Schraudolph constants (calibrated: +-0.82% minimax exp error).
# p ~= S(t - 1024) + W * S(t + 512) with t = scores*A_SCALE + 15360 - c_opt,
# where S(u) = fp16_bits(int16(u)). B1 = bias for passA (fp16-exact),
# MASKED_B forces int16 saturation at -32768 = fp16 -0.0.
A_SCALE = 128 * float(np.log2(np.e))   # 0.125 (dk scale) * 1024 * log2(e)
B1 = 14256.0
W_PAIR = 0.3655
MASKED_B = -50000.0

f32 = mybir.dt.float32
f16 = mybir.dt.float16
i16 = mybir.dt.int16
FT = mybir.ActivationFunctionType


def build_nc(hpc=HPC, s=S, loop_n=None, ablate=(), loop_stagger=False,
             ndve=NDVE):
    """Build the per-core Bass program (identical on all 8 cores).

    loop_n: if set, wrap the whole body in an on-device For_i loop that
    recomputes the same output loop_n times — a perf-measurement rig that
    lets wall-clock deltas between two loop_n values cancel host/RPC
    overheads (this container has no NTFF profile path).

    ablate: perf-debug only — subset of {"qk", "act", "dexp", "mask", "pv",
    "tail"} to skip emitting, isolating per-engine throughput on HW. Output
    is garbage when non-empty.
    """
    nsq = s // SQ
    nck = s // CK
    if nck != 16:
        ndve = 0  # debug builds: plain per-chunk ACT path
    nact = nck - ndve
    if nck == 16:
        act_groups = [(c, min(3, nact - c)) for c in range(0, nact, 3)]
        dve_groups = [(c, min(3, nck - c)) for c in range(nact, nck, 3)]
    else:
        act_groups = [(c, 1) for c in range(nck)]
        dve_groups = []
    ablate = set(ablate)

    nc = bacc.Bacc("TRN2", target_bir_lowering=False, debug=False)

    qt_d = nc.dram_tensor("qt", [hpc, D, s], f16, kind="ExternalInput")
    kt_d = nc.dram_tensor("kt", [hpc, D, s], f16, kind="ExternalInput")
    vp_d = nc.dram_tensor("vp", [hpc, CK, nck * VW], f16, kind="ExternalInput")
    mk_d = nc.dram_tensor("mk", [nsq, CK, nck * SQ], f16, kind="ExternalInput")
    if ndve:
        mb_d = nc.dram_tensor("mb", [nsq, CK, ndve * SQ], f16,
                              kind="ExternalInput")
    o_d = nc.dram_tensor("o", [hpc, nsq, VW, SQ], f32, kind="ExternalOutput")

    with tile.TileContext(nc) as tc:
        if ablate:
            tc.race_detector_enabled = False
        with (
            tc.tile_pool(name="heads", bufs=hpc) as head_pool,
            tc.tile_pool(name="mask", bufs=nsq) as mask_pool,
            tc.tile_pool(name="pt", bufs=2) as pt_pool,
            tc.tile_pool(name="u16", bufs=2) as u_pool,
            tc.tile_pool(name="tail", bufs=2) as tail_pool,
            tc.tile_pool(name="qk_ps", bufs=2, space="PSUM") as qk_pool,
            tc.tile_pool(name="o_ps", bufs=2, space="PSUM") as o_pool,
        ):
            qt_t, kt_t, vp_t = [], [], []
            for h in range(hpc):
                q_t = head_pool.tile([128, s], f16, name=f"qt_sb{h}", tag="qt")
                k_t = head_pool.tile([128, s], f16, name=f"kt_sb{h}", tag="kt")
                v_t = head_pool.tile([CK, nck * VW], f16, name=f"vp_sb{h}", tag="vp")
                # Q^T/K^T live duplicated in both partition halves so the two
                # row-packed K=64 matmuls can run concurrently on the PE.
                nc.sync.dma_start(out=q_t[0:D, :], in_=qt_d[h, :, :])
                nc.sync.dma_start(out=q_t[D:128, :], in_=qt_d[h, :, :])
                nc.sync.dma_start(out=k_t[0:D, :], in_=kt_d[h, :, :])
                nc.sync.dma_start(out=k_t[D:128, :], in_=kt_d[h, :, :])
                nc.sync.dma_start(out=v_t[:, :], in_=vp_d[h, :, :])
                qt_t.append(q_t)
                kt_t.append(k_t)
                vp_t.append(v_t)

            # The whole 0/1 mask (and the Schraudolph bias mask) fit in SBUF —
            # load once, outside any measurement loop.
            mk_t = {}     # sqb -> mask tile [128, nck*SQ] (chunk-major columns)
            mb_t = {}     # sqb -> passA bias tile [128, ndve*SQ]
            for sqb in range(nsq):
                mk = mask_pool.tile([CK, nck * SQ], f16, name=f"mk_sb{sqb}",
                                    tag="mk")
                nc.sync.dma_start(out=mk[:, :], in_=mk_d[sqb, :, :])
                mk_t[sqb] = mk
                if ndve:
                    mb = mask_pool.tile([CK, ndve * SQ], f16,
                                        name=f"mb_sb{sqb}", tag="mb")
                    nc.sync.dma_start(out=mb[:, :], in_=mb_d[sqb, :, :])
                    mb_t[sqb] = mb

            pt_t = {}     # (sqb, h) -> p^T tile [128, nck*SQ] fp16
            o_ps = {}     # (sqb, h) -> PSUM accumulator [VW, SQ]

            def emit_qk(qk, sqb, h, c0, n):
                """QK matmuls for chunks [c0, c0+n) into psum tile qk."""
                for j in range(n):
                    c = c0 + j
                    # Row-group alternation must be strict across the WHOLE
                    # chunk sequence (c % 2): only back-to-back (0, 64) pairs
                    # stream concurrently on the PE (128.5 ns/MM vs 359).
                    bp = 64 * (c % 2)
                    nc.tensor.matmul(
                        qk[:, j * SQ:(j + 1) * SQ],
                        lhsT=kt_t[h][bp:bp + D, c * CK:(c + 1) * CK],
                        rhs=qt_t[h][bp:bp + D, sqb * SQ:(sqb + 1) * SQ],
                        start=True,
                        stop=True,
                        tile_position=(bp, 0),
                    )

            def emit_act_group(sqb, h, c0, n):
                """QK + ACT exp for chunks [c0, c0+n)."""
                qk = None
                if "qk" not in ablate:
                    qk = qk_pool.tile([128, n * SQ], f32,
                                      name=f"qk_{sqb}_{h}_{c0}", tag="qk",
                                      padded_shape=[128, 3 * SQ])
                    emit_qk(qk, sqb, h, c0, n)
                pt = pt_t[(sqb, h)]
                lo = c0 * SQ
                hi = (c0 + n) * SQ
                if "act" not in ablate:
                    act_in = qk[:, :] if qk is not None else mk_t[sqb][:, lo:hi]
                    nc.scalar.activation(pt[:, lo:hi], act_in, FT.Exp, scale=0.125)

            def emit_dve(sqb, h):
                """QK + pair-Schraudolph exp for the last ndve chunks."""
                pt = pt_t[(sqb, h)]
                u1 = u_pool.tile([128, ndve * SQ], i16, name=f"u1_{sqb}_{h}",
                                 tag="u1")
                u2 = u_pool.tile([128, ndve * SQ], i16, name=f"u2_{sqb}_{h}",
                                 tag="u2")
                for c0, n in dve_groups:
                    qk = None
                    if "qk" not in ablate:
                        qk = qk_pool.tile([128, n * SQ], f32,
                                          name=f"qk_{sqb}_{h}_{c0}", tag="qk",
                                          padded_shape=[128, 3 * SQ])
                        emit_qk(qk, sqb, h, c0, n)
                    if "dexp" in ablate:
                        continue
                    off = (c0 - nact) * SQ
                    a_in = qk[:, :] if qk is not None else \
                        mb_t[sqb][:, off:off + n * SQ]
                    nc.vector.scalar_tensor_tensor(
                        u1[:, off:off + n * SQ], a_in, A_SCALE,
                        mb_t[sqb][:, off:off + n * SQ],
                        op0=mybir.AluOpType.mult, op1=mybir.AluOpType.add)
                if "dexp" in ablate:
                    return
                nc.vector.tensor_scalar(
                    u2[:, :], u1[:, :], 1536.0, 0.0,
                    op0=mybir.AluOpType.add, op1=mybir.AluOpType.max)
                nc.vector.scalar_tensor_tensor(
                    pt[:, nact * SQ:nck * SQ], u2[:, :].bitcast(f16), W_PAIR,
                    u1[:, :].bitcast(f16),
                    op0=mybir.AluOpType.mult, op1=mybir.AluOpType.add)

            def emit_mask(sqb, h, clo, chi):
                """Apply the 0/1 keep-mask to chunk cols [clo, chi) of p^T
                in one DVE pass (per-call overhead is ~0.4us; batch big)."""
                if "mask" in ablate or clo >= chi:
                    return
                pt = pt_t[(sqb, h)]
                lo, hi = clo * SQ, chi * SQ
                nc.vector.tensor_tensor(
                    pt[:, lo:hi], pt[:, lo:hi], mk_t[sqb][:, lo:hi],
                    op=mybir.AluOpType.mult,
                )

            def emit_pv(sqb, h, clo, chi):
                """PV matmuls for chunks [clo, chi), accumulating."""
                if "pv" in ablate:
                    return
                pt = pt_t[(sqb, h)]
                if pt is None:
                    pt = mk_t[sqb]  # stand-in written tile for PE-only ablations
                ops = o_ps[(sqb, h)]
                for c in range(clo, chi):
                    # Two N=256 matmuls per chunk hide the per-chunk
                    # LDWEIGHTS (128.3 ns/MM vs 336 at N=512).  start=True
                    # clears the WHOLE PSUM bank, so only the very first
                    # matmul of the accumulation may carry it.
                    for hf in range(2):
                        nc.tensor.matmul(
                            ops[:, hf * 256:(hf + 1) * 256],
                            lhsT=vp_t[h][:, c * VW:c * VW + VW],
                            rhs=pt[:, c * SQ + hf * 256:c * SQ + (hf + 1) * 256],
                            start=(c == 0 and hf == 0),
                            stop=(c == nck - 1),
                        )

            def emit_tail(sqb, h):
                """Evacuate O^T' (unnormalized + Z row) and store."""
                if "tail" in ablate:
                    return
                ops = o_ps[(sqb, h)]
                ot = tail_pool.tile([VW, SQ], f32, name=f"ot_{sqb}_{h}", tag="ot")
                nc.vector.tensor_copy(ot[:, :], ops[:, :])
                nc.sync.dma_start(out=o_d[h, sqb, :, :], in_=ot[:, :])

            # Half-stage software pipeline over (sqb, h, half): half k's
            # QK/exp/mask stream overlaps the PV matmuls of half k-2 (one
            # full stage earlier), so the in-order PE queue never stalls on
            # the ACT/DVE work of the half just emitted.
            ng = len(act_groups)
            halves = [(0, act_groups[:(ng + 1) // 2], False),
                      (1, act_groups[(ng + 1) // 2:], bool(ndve))]

            def emit_front(sqb, h, hf):
                _, groups, dve = halves[hf]
                if hf == 0:
                    if not ({"act", "dexp", "mask"} <= ablate):
                        pt_t[(sqb, h)] = pt_pool.tile(
                            [128, nck * SQ], f16, name=f"pt_{sqb}_{h}",
                            tag="pt")
                    else:
                        pt_t[(sqb, h)] = None
                    if "pv" not in ablate:
                        o_ps[(sqb, h)] = o_pool.tile(
                            [VW, SQ], f32, name=f"ops_{sqb}_{h}", tag="ops")
                for c0, n in groups:
                    emit_act_group(sqb, h, c0, n)
                if groups:
                    emit_mask(sqb, h, groups[0][0],
                              groups[-1][0] + groups[-1][1])
                if dve:
                    emit_dve(sqb, h)

            def emit_back(sqb, h, hf):
                _, groups, dve = halves[hf]
                clo = groups[0][0] if groups else nact
                chi = nck if dve else groups[-1][0] + groups[-1][1]
                emit_pv(sqb, h, clo, chi)
                if hf == len(halves) - 1:
                    emit_tail(sqb, h)

            def emit_all():
                hvs = [(sqb, h, hf)
                       for sqb in range(nsq) for h in range(hpc)
                       for hf in range(len(halves))]
                for k, hv in enumerate(hvs):
                    emit_front(*hv)
                    if k >= 2:
                        emit_back(*hvs[k - 2])
                for hv in hvs[-2:]:
                    emit_back(*hv)

            if loop_n is None:
                emit_all()
            else:
                hints = (mybir.EngineType.PE, mybir.EngineType.Activation,
                         mybir.EngineType.DVE)
                with tc.For_i(0, loop_n, 1, hint_engines=hints,
                              staggered_reset=bool(loop_stagger)):
                    emit_all()

    nc.finalize()
    return nc


def shard_inputs(K, Q, V, mask, hpc=HPC, s=S, n_cores=N_CORES, ndve=NDVE):
    """Full inputs -> per-core in_maps with device-friendly host layouts."""
    nsq = s // SQ
    nck = s // CK
    if nck != 16:
        ndve = 0
    nact = nck - ndve
    n_units = n_cores * hpc
    Kf = np.asarray(K, np.float32).reshape(n_units, s, D)
    Qf = np.asarray(Q, np.float32).reshape(n_units, s, D)
    Vf = np.asarray(V, np.float32).reshape(n_units, s, D)
    keepT = (~np.asarray(mask).reshape(s, s)).T  # [k, q], True = attend
    keep4 = keepT.reshape(nck, CK, nsq, SQ).transpose(2, 1, 0, 3)
    mk_host = np.ascontiguousarray(
        keep4.astype(np.float16).reshape(nsq, CK, nck * SQ))
    mb_host = None
    if ndve:
        mb_host = np.where(keep4[:, :, nact:, :], np.float16(B1),
                           np.float16(MASKED_B))
        mb_host = np.ascontiguousarray(
            mb_host.astype(np.float16).reshape(nsq, CK, ndve * SQ))
    in_maps = []
    for c in range(n_cores):
        sl = slice(c * hpc, (c + 1) * hpc)
        qt = np.ascontiguousarray(Qf[sl].transpose(0, 2, 1)).astype(np.float16)
        kt = np.ascontiguousarray(Kf[sl].transpose(0, 2, 1)).astype(np.float16)
        vp = np.zeros((hpc, s, VW), np.float16)
        vp[:, :, :D] = Vf[sl]
        vp[:, :, D] = 1.0
        vp = np.ascontiguousarray(
            vp.reshape(hpc, nck, CK, VW).transpose(0, 2, 1, 3)
            .reshape(hpc, CK, nck * VW)
        )
        im = {"qt": qt, "kt": kt, "vp": vp, "mk": mk_host}
        if ndve:
            im["mb"] = mb_host
        in_maps.append(im)
    return in_maps


_NC_CACHE = {}


def _get_nc():
    if "nc" not in _NC_CACHE:
        _NC_CACHE["nc"] = build_nc()
    return _NC_CACHE["nc"]


def run_sharded(in_maps, trace=False, **kwargs):
    return run_bass_kernel_spmd(
        _get_nc(), in_maps, core_ids=list(range(N_CORES)), trace=trace, **kwargs
    )


def unshard_output(per_core_raw, hpc=HPC, s=S):
    """[hpc, nsq, VW, SQ] raw blocks per core -> [n*hpc, s, D] normalized.

    Row D of each block is the softmax denominator Z; dividing and
    transposing here is O(S*D) host work (same order as unsharding).
    """
    n = len(per_core_raw)
    out = np.empty((n * hpc, s, D), np.float32)
    for c, o in enumerate(per_core_raw):
        ot = o[:, :, :D, :] / o[:, :, D:D + 1, :]   # [hpc, nsq, D, SQ]
        out[c * hpc:(c + 1) * hpc] = (
            ot.transpose(0, 1, 3, 2).reshape(hpc, s, D))
    return out


def assemble_output(results):
    out = unshard_output([results[c]["o"] for c in range(N_CORES)])
    return out.reshape(B, H, S, D)


def kernel(K, Q, V, mask):
    in_maps = shard_inputs(K, Q, V, mask)
    res = run_sharded(in_maps)
    return assemble_output(res.results)


# revision 8
# speedup vs baseline: 1.7209x; 1.5805x over previous
"""Multi-head attention (B=2, H=16, S=2048, D=64) on 8 trn2 NeuronCores.

Sharding: the 32 (b, h) head-units are split 4-per-core (head/data parallel,
no cross-core comms).  Per core, for each head:

  scoresT[k, q] = sum_d K[k, d] Q[q, d] / 8        (PE, contract=64, row-packed 2x)
  pT[k, q]      = exp(scoresT) * keep01T[k, q]     (split ACT / DVE, see below)
  OT'[m, q]     = sum_k V'[k, m] pT[k, q]          (PE, V' = [V | ones] so row 64
                                                    of OT' is the softmax denom Z)
  out[q, d]     = OT'[d, q] / OT'[64, q]           (host-side: O(S*D) divide +
                                                    transpose while unsharding)

Working in the transposed-score layout means softmax needs no reductions at
all (Z rides along in the PV matmul) and no S x S transposes anywhere.

exp+mask engine split: ACT's 1 elem/cycle/lane rate makes the exp of all
S*S scores the systemic bottleneck, so the last NDVE k-chunks bypass ACT and
are exponentiated on the DVE with a two-sample Schraudolph bit-trick:
  passA: u1 = int16(scores*A + maskbias)  (scalar_tensor_tensor from PSUM;
         maskbias = B1 for attend / -50000 for masked -> saturates to -32768,
         whose fp16 bit-pattern is -0.0, i.e. the mask is folded in for free)
  passC: u2 = max(u1 + 1536, 0)           (tensor_scalar, 4x mode; masked
         lanes land at exactly +0.0)
  passD: p = f16view(u2) * W + f16view(u1)  (pair-average cancels the primary
         mantissa-interpolation ripple: +-0.8% exp error, ~4e-3 end-to-end)

PE shape rules (measured): QK row-group alternation must be strict (c % 2)
for concurrent streaming of packed contract-64 pairs (128.5 ns/MM); PV runs
as two N=256 matmuls per chunk to hide per-chunk LDWEIGHTS (128.3 ns/MM).

Host-side (numpy, not on the critical HW path): Q/K are passed pre-transposed
per head as [64, S]; V is passed chunk-interleaved fp16 with the ones column
appended; the shared mask is passed transposed as a 0/1 fp16 matrix plus a
Schraudolph bias matrix for the DVE chunks.
"""

import numpy as np

import concourse.bass as bass  # noqa: F401  (engine types resolve through nc)
import concourse.mybir as mybir
import concourse.tile as tile
from concourse import bacc
from concourse.bass_utils import run_bass_kernel_spmd

B, H, S, D = 2, 16, 2048, 64
N_CORES = 8
HPC = (B * H) // N_CORES  # heads per core

SQ = 512        # query-block width (one fp32 PSUM bank)
CK = 128        # key-chunk height (PSUM partition dim)
NDVE = 4        # trailing k-chunks exponentiated on the DVE instead of ACT
# Key chunks per exp group: 3-bank [128, 1536] PSUM groups maximize the ACT
# call size (per-ACTIVATE overhead is ~0.4us on HW) within the 8-bank budget
# (2x 3-bank qk slots + 2x 1-bank PV accumulators).
VW = D + 2      # V' width: 64 V columns + ones column + pad (66)

# Two-sample Schraudolph constants (calibrated: +-0.82% minimax exp error).
# p ~= S(t - 1024) + W * S(t + 512) with t = scores*A_SCALE + 15360 - c_opt,
# where S(u) = fp16_bits(int16(u)). B1 = bias for passA (fp16-exact),
# MASKED_B forces int16 saturation at -32768 = fp16 -0.0.
A_SCALE = 128 * float(np.log2(np.e))   # 0.125 (dk scale) * 1024 * log2(e)
B1 = 14256.0
W_PAIR = 0.3655
MASKED_B = -50000.0

f32 = mybir.dt.float32
f16 = mybir.dt.float16
i16 = mybir.dt.int16
FT = mybir.ActivationFunctionType


def build_nc(hpc=HPC, s=S, loop_n=None, ablate=(), loop_stagger=False,
             ndve=NDVE):
    """Build the per-core Bass program (identical on all 8 cores).

    loop_n: if set, wrap the whole body in an on-device For_i loop that
    recomputes the same output loop_n times — a perf-measurement rig that
    lets wall-clock deltas between two loop_n values cancel host/RPC
    overheads (this container has no NTFF profile path).

    ablate: perf-debug only — subset of {"qk", "act", "dexp", "mask", "pv",
    "tail"} to skip emitting, isolating per-engine throughput on HW. Output
    is garbage when non-empty.
    """
    nsq = s // SQ
    nck = s // CK
    if nck != 16:
        ndve = 0  # debug builds: plain per-chunk ACT path
    nact = nck - ndve
    if nck == 16:
        act_groups = [(c, min(3, nact - c)) for c in range(0, nact, 3)]
        dve_groups = [(c, min(3, nck - c)) for c in range(nact, nck, 3)]
    else:
        act_groups = [(c, 1) for c in range(nck)]
        dve_groups = []
    ablate = set(ablate)

    nc = bacc.Bacc("TRN2", target_bir_lowering=False, debug=False)

    qt_d = nc.dram_tensor("qt", [hpc, D, s], f16, kind="ExternalInput")
    kt_d = nc.dram_tensor("kt", [hpc, D, s], f16, kind="ExternalInput")
    vp_d = nc.dram_tensor("vp", [hpc, CK, nck * VW], f16, kind="ExternalInput")
    mk_d = nc.dram_tensor("mk", [nsq, CK, nck * SQ], f16, kind="ExternalInput")
    if ndve:
        mb_d = nc.dram_tensor("mb", [nsq, CK, ndve * SQ], f16,
                              kind="ExternalInput")
    o_d = nc.dram_tensor("o", [hpc, nsq, VW, SQ], f32, kind="ExternalOutput")

    with tile.TileContext(nc) as tc:
        if ablate:
            tc.race_detector_enabled = False
        with (
            tc.tile_pool(name="heads", bufs=hpc) as head_pool,
            tc.tile_pool(name="mask", bufs=nsq) as mask_pool,
            tc.tile_pool(name="pt", bufs=2) as pt_pool,
            tc.tile_pool(name="u16", bufs=2) as u_pool,
            tc.tile_pool(name="tail", bufs=2) as tail_pool,
            tc.tile_pool(name="qk_ps", bufs=2, space="PSUM") as qk_pool,
            tc.tile_pool(name="o_ps", bufs=2, space="PSUM") as o_pool,
        ):
            qt_t, kt_t, vp_t = [], [], []
            for h in range(hpc):
                q_t = head_pool.tile([128, s], f16, name=f"qt_sb{h}", tag="qt")
                k_t = head_pool.tile([128, s], f16, name=f"kt_sb{h}", tag="kt")
                v_t = head_pool.tile([CK, nck * VW], f16, name=f"vp_sb{h}", tag="vp")
                # Q^T/K^T live duplicated in both partition halves so the two
                # row-packed K=64 matmuls can run concurrently on the PE.
                nc.sync.dma_start(out=q_t[0:D, :], in_=qt_d[h, :, :])
                nc.sync.dma_start(out=q_t[D:128, :], in_=qt_d[h, :, :])
                nc.sync.dma_start(out=k_t[0:D, :], in_=kt_d[h, :, :])
                nc.sync.dma_start(out=k_t[D:128, :], in_=kt_d[h, :, :])
                nc.sync.dma_start(out=v_t[:, :], in_=vp_d[h, :, :])
                qt_t.append(q_t)
                kt_t.append(k_t)
                vp_t.append(v_t)

            # The whole 0/1 mask (and the Schraudolph bias mask) fit in SBUF —
            # load once, outside any measurement loop.
            mk_t = {}     # sqb -> mask tile [128, nck*SQ] (chunk-major columns)
            mb_t = {}     # sqb -> passA bias tile [128, ndve*SQ]
            for sqb in range(nsq):
                mk = mask_pool.tile([CK, nck * SQ], f16, name=f"mk_sb{sqb}",
                                    tag="mk")
                nc.sync.dma_start(out=mk[:, :], in_=mk_d[sqb, :, :])
                mk_t[sqb] = mk
                if ndve:
                    mb = mask_pool.tile([CK, ndve * SQ], f16,
                                        name=f"mb_sb{sqb}", tag="mb")
                    nc.sync.dma_start(out=mb[:, :], in_=mb_d[sqb, :, :])
                    mb_t[sqb] = mb

            pt_t = {}     # (sqb, h) -> p^T tile [128, nck*SQ] fp16
            o_ps = {}     # (sqb, h) -> PSUM accumulator [VW, SQ]

            def emit_qk(qk, sqb, h, c0, n):
                """QK matmuls for chunks [c0, c0+n) into psum tile qk."""
                for j in range(n):
                    c = c0 + j
                    # Row-group alternation must be strict across the WHOLE
                    # chunk sequence (c % 2): only back-to-back (0, 64) pairs
                    # stream concurrently on the PE (128.5 ns/MM vs 359).
                    bp = 64 * (c % 2)
                    nc.tensor.matmul(
                        qk[:, j * SQ:(j + 1) * SQ],
                        lhsT=kt_t[h][bp:bp + D, c * CK:(c + 1) * CK],
                        rhs=qt_t[h][bp:bp + D, sqb * SQ:(sqb + 1) * SQ],
                        start=True,
                        stop=True,
                        tile_position=(bp, 0),
                    )

            def emit_act_group(sqb, h, c0, n):
                """QK + ACT exp for chunks [c0, c0+n)."""
                qk = None
                if "qk" not in ablate:
                    qk = qk_pool.tile([128, n * SQ], f32,
                                      name=f"qk_{sqb}_{h}_{c0}", tag="qk",
                                      padded_shape=[128, 3 * SQ])
                    emit_qk(qk, sqb, h, c0, n)
                pt = pt_t[(sqb, h)]
                lo = c0 * SQ
                hi = (c0 + n) * SQ
                if "act" not in ablate:
                    act_in = qk[:, :] if qk is not None else mk_t[sqb][:, lo:hi]
                    nc.scalar.activation(pt[:, lo:hi], act_in, FT.Exp, scale=0.125)

            def emit_dve(sqb, h):
                """QK + pair-Schraudolph exp for the last ndve chunks."""
                pt = pt_t[(sqb, h)]
                u1 = u_pool.tile([128, ndve * SQ], i16, name=f"u1_{sqb}_{h}",
                                 tag="u1")
                u2 = u_pool.tile([128, ndve * SQ], i16, name=f"u2_{sqb}_{h}",
                                 tag="u2")
                for c0, n in dve_groups:
                    qk = None
                    if "qk" not in ablate:
                        qk = qk_pool.tile([128, n * SQ], f32,
                                          name=f"qk_{sqb}_{h}_{c0}", tag="qk",
                                          padded_shape=[128, 3 * SQ])
                        emit_qk(qk, sqb, h, c0, n)
                    if "dexp" in ablate:
                        continue
                    off = (c0 - nact) * SQ
                    a_in = qk[:, :] if qk is not None else \
                        mb_t[sqb][:, off:off + n * SQ]
                    nc.vector.scalar_tensor_tensor(
                        u1[:, off:off + n * SQ], a_in, A_SCALE,
                        mb_t[sqb][:, off:off + n * SQ],
                        op0=mybir.AluOpType.mult, op1=mybir.AluOpType.add)
                if "dexp" in ablate:
                    return
                nc.vector.tensor_scalar(
                    u2[:, :], u1[:, :], 1536.0, 0.0,
                    op0=mybir.AluOpType.add, op1=mybir.AluOpType.max)
                nc.vector.scalar_tensor_tensor(
                    pt[:, nact * SQ:nck * SQ], u2[:, :].bitcast(f16), W_PAIR,
                    u1[:, :].bitcast(f16),
                    op0=mybir.AluOpType.mult, op1=mybir.AluOpType.add)

            def emit_mask(sqb, h, clo, chi):
                """Apply the 0/1 keep-mask to chunk cols [clo, chi) of p^T
                in one DVE pass (per-call overhead is ~0.4us; batch big)."""
                if "mask" in ablate or clo >= chi:
                    return
                pt = pt_t[(sqb, h)]
                lo, hi = clo * SQ, chi * SQ
                nc.vector.tensor_tensor(
                    pt[:, lo:hi], pt[:, lo:hi], mk_t[sqb][:, lo:hi],
                    op=mybir.AluOpType.mult,
                )

            def emit_pv(sqb, h, clo, chi):
                """PV matmuls for chunks [clo, chi), accumulating."""
                if "pv" in ablate:
                    return
                pt = pt_t[(sqb, h)]
                if pt is None:
                    pt = mk_t[sqb]  # stand-in written tile for PE-only ablations
                ops = o_ps[(sqb, h)]
                for c in range(clo, chi):
                    # Two N=256 matmuls per chunk hide the per-chunk
                    # LDWEIGHTS (128.3 ns/MM vs 336 at N=512).  start=True
                    # clears the WHOLE PSUM bank, so only the very first
                    # matmul of the accumulation may carry it.
                    for hf in range(2):
                        nc.tensor.matmul(
                            ops[:, hf * 256:(hf + 1) * 256],
                            lhsT=vp_t[h][:, c * VW:c * VW + VW],
                            rhs=pt[:, c * SQ + hf * 256:c * SQ + (hf + 1) * 256],
                            start=(c == 0 and hf == 0),
                            stop=(c == nck - 1),
                        )

            def emit_tail(sqb, h):
                """Evacuate O^T' (unnormalized + Z row) and store."""
                if "tail" in ablate:
                    return
                ops = o_ps[(sqb, h)]
                ot = tail_pool.tile([VW, SQ], f32, name=f"ot_{sqb}_{h}", tag="ot")
                nc.vector.tensor_copy(ot[:, :], ops[:, :])
                nc.sync.dma_start(out=o_d[h, sqb, :, :], in_=ot[:, :])

            # Stage pipeline over (sqb, h, stage): each stage allocates at
            # most TWO qk psum slots (= the whole ping-pong budget), so the
            # next stage's first QK matmul waits only on the exp of the
            # stage before last.  PV batches of stage k-2 are emitted
            # BETWEEN the QK bursts, keeping the in-order PE queue fed
            # while ACT/DVE drain the just-filled slots.
            if ndve:
                stages = [("act", act_groups[:2]), ("act", act_groups[2:]),
                          ("dve", dve_groups)]
            else:
                stages = [("act", act_groups[i:i + 2])
                          for i in range(0, len(act_groups), 2)]
            s_rng = []
            for kind, groups in stages:
                clo = groups[0][0]
                chi = groups[-1][0] + groups[-1][1]
                s_rng.append((clo, chi))

            def emit_front(sqb, h, st):
                kind, groups = stages[st]
                if st == 0:
                    if not ({"act", "dexp", "mask"} <= ablate):
                        pt_t[(sqb, h)] = pt_pool.tile(
                            [128, nck * SQ], f16, name=f"pt_{sqb}_{h}",
                            tag="pt")
                    else:
                        pt_t[(sqb, h)] = None
                    if "pv" not in ablate:
                        o_ps[(sqb, h)] = o_pool.tile(
                            [VW, SQ], f32, name=f"ops_{sqb}_{h}", tag="ops")
                if kind == "act":
                    for c0, n in groups:
                        emit_act_group(sqb, h, c0, n)
                else:
                    # The Schraudolph passes read the qk psum slots, so they
                    # go FIRST on the DVE; the mask multiplies only touch
                    # SBUF and run while the PE refills the freed slots.
                    emit_dve(sqb, h)
                    for pst in range(st):
                        emit_mask(sqb, h, *s_rng[pst])
                if kind == "act" and len(stages) == st + 1:
                    # no dve stage (debug builds): mask inline
                    emit_mask(sqb, h, s_rng[0][0], s_rng[st][1])

            def emit_back(sqb, h, st):
                emit_pv(sqb, h, *s_rng[st])
                if st == len(stages) - 1:
                    emit_tail(sqb, h)

            def emit_all():
                hvs = [(sqb, h, st)
                       for sqb in range(nsq) for h in range(hpc)
                       for st in range(len(stages))]
                for k, hv in enumerate(hvs):
                    emit_front(*hv)
                    if k >= 2:
                        emit_back(*hvs[k - 2])
                for hv in hvs[-2:]:
                    emit_back(*hv)

            if loop_n is None:
                emit_all()
            else:
                hints = (mybir.EngineType.PE, mybir.EngineType.Activation,
                         mybir.EngineType.DVE)
                with tc.For_i(0, loop_n, 1, hint_engines=hints,
                              staggered_reset=bool(loop_stagger)):
                    emit_all()

    nc.finalize()
    return nc


def shard_inputs(K, Q, V, mask, hpc=HPC, s=S, n_cores=N_CORES, ndve=NDVE):
    """Full inputs -> per-core in_maps with device-friendly host layouts."""
    nsq = s // SQ
    nck = s // CK
    if nck != 16:
        ndve = 0
    nact = nck - ndve
    n_units = n_cores * hpc
    Kf = np.asarray(K, np.float32).reshape(n_units, s, D)
    Qf = np.asarray(Q, np.float32).reshape(n_units, s, D)
    Vf = np.asarray(V, np.float32).reshape(n_units, s, D)
    keepT = (~np.asarray(mask).reshape(s, s)).T  # [k, q], True = attend
    keep4 = keepT.reshape(nck, CK, nsq, SQ).transpose(2, 1, 0, 3)
    mk_host = np.ascontiguousarray(
        keep4.astype(np.float16).reshape(nsq, CK, nck * SQ))
    mb_host = None
    if ndve:
        mb_host = np.where(keep4[:, :, nact:, :], np.float16(B1),
                           np.float16(MASKED_B))
        mb_host = np.ascontiguousarray(
            mb_host.astype(np.float16).reshape(nsq, CK, ndve * SQ))
    in_maps = []
    for c in range(n_cores):
        sl = slice(c * hpc, (c + 1) * hpc)
        qt = np.ascontiguousarray(Qf[sl].transpose(0, 2, 1)).astype(np.float16)
        kt = np.ascontiguousarray(Kf[sl].transpose(0, 2, 1)).astype(np.float16)
        vp = np.zeros((hpc, s, VW), np.float16)
        vp[:, :, :D] = Vf[sl]
        vp[:, :, D] = 1.0
        vp = np.ascontiguousarray(
            vp.reshape(hpc, nck, CK, VW).transpose(0, 2, 1, 3)
            .reshape(hpc, CK, nck * VW)
        )
        im = {"qt": qt, "kt": kt, "vp": vp, "mk": mk_host}
        if ndve:
            im["mb"] = mb_host
        in_maps.append(im)
    return in_maps


_NC_CACHE = {}


def _get_nc():
    if "nc" not in _NC_CACHE:
        _NC_CACHE["nc"] = build_nc()
    return _NC_CACHE["nc"]


def run_sharded(in_maps, trace=False, **kwargs):
    return run_bass_kernel_spmd(
        _get_nc(), in_maps, core_ids=list(range(N_CORES)), trace=trace, **kwargs
    )


def unshard_output(per_core_raw, hpc=HPC, s=S):
    """[hpc, nsq, VW, SQ] raw blocks per core -> [n*hpc, s, D] normalized.

    Row D of each block is the softmax denominator Z; dividing and
    transposing here is O(S*D) host work (same order as unsharding).
    """
    n = len(per_core_raw)
    out = np.empty((n * hpc, s, D), np.float32)
    for c, o in enumerate(per_core_raw):
        ot = o[:, :, :D, :] / o[:, :, D:D + 1, :]   # [hpc, nsq, D, SQ]
        out[c * hpc:(c + 1) * hpc] = (
            ot.transpose(0, 1, 3, 2).reshape(hpc, s, D))
    return out


def assemble_output(results):
    out = unshard_output([results[c]["o"] for c in range(N_CORES)])
    return out.reshape(B, H, S, D)


def kernel(K, Q, V, mask):
    in_maps = shard_inputs(K, Q, V, mask)
    res = run_sharded(in_maps)
    return assemble_output(res.results)
